# revision 15
# baseline (speedup 1.0000x reference)
"""ConnectivityLoss kernel for Trainium2 (Bass/Tile), 8-core data-parallel.

Math: the reference's 32-step 3x3 max-dilation chain cancels algebraically.
For binary maps, dilation D(x) >= x pointwise (3x3 SAME window contains the
center), so pred_bin * D32(gt_bin) * gt_bin * D32(pred_bin) == pred_bin * gt_bin
for every pixel: whenever both center bits are 1 the two dilations are 1 at
that pixel too, and otherwise the product is 0 regardless.  Hence

    match[b,k,i,j] = (alpha_pred > t_k) * (alpha_gt > t_k)
                   = (min(alpha_pred, alpha_gt) > t_k)

    err_px = (101 - cnt) / 101      with cnt = #{k in 0..100 : t_k < m},
                                    m = min(alpha_pred, alpha_gt)
    loss   = sum(err_px * [trimap == 128]) / (sum([trimap == 128]) + 1e-8)

cnt is evaluated per pixel with an exactly-rounded two-scale trick instead of
101 compares (t_k = RN(k * fp32(0.01)), the jnp.arange values):

    v128 = m * 128                  exact (power of two)
    v    = v128 * 0.78125           = RN(m * 100)   (0.78125 = 100/128 exact)
    r    = RN(RN(v + 2^23) - 2^23)  = round-to-nearest-int(v), candidate bin
    u    = r * 1.28                 = RN(r * (fp32(0.01)*128)) = t_r * 128
                                      exactly (x128 commutes with rounding)
    cnt  = r + [v128 > u]           since t_r < m <=> u < v128 exactly, and
                                    r is within 0.5 of m*100 so the count can
                                    only be r or r+1.

Verified exhaustively against the reference predicate for ALL 2^30 fp32
values in [0,1): zero mismatches.  m = 0 (unmasked pixels zeroed by the mask
multiply) gives r = 0, u = 0, [0 > 0] = 0 -> cnt = 0 as required.

Sharding: data-parallel over flattened B*H*W pixels, 8 equal contiguous
shards of 32768 = 128x256 elements; each core reduces its shard to
per-partition partial sums (sum r, sum g, sum mask), host combines scalars.
"""

import numpy as np

N_CORES = 8
P = 128          # SBUF partitions
F = 256          # free dim; per-core shard = P*F = 32768 pixels
TOTAL = 4 * 1 * 256 * 256

MAGIC = 8388608.0                       # 2^23
C128 = float(np.float32(0.01) * np.float32(128.0))   # 1.28 in fp32, exact
SCALE = 0.78125                          # 100/128, exact in fp32

_CACHE = {}


def _build():
    import concourse.bass as bass
    import concourse.tile as tile
    from concourse import mybir

    f32 = mybir.dt.float32
    i32 = mybir.dt.int32
    Op = mybir.AluOpType

    nc = bass.Bass(
        "TRN2",
        target_bir_lowering=False,
        debug=False,
        enable_asserts=False,
        num_devices=N_CORES,
    )
    packed = nc.dram_tensor("packed", [P, 3 * F], i32, kind="ExternalInput")
    out = nc.dram_tensor("stats", [P, 3], f32, kind="ExternalOutput")

    with tile.TileContext(nc) as tc:
        with tc.tile_pool(name="pool", bufs=1) as pool:
            tin = pool.tile([P, 3 * F], i32)
            nc.sync.dma_start(tin[:], packed[:])
            tp = tin[:, 0:F].bitcast(f32)
            tg = tin[:, F:2 * F].bitcast(f32)
            tt = tin[:, 2 * F:3 * F]

            mask = pool.tile([P, F], f32)
            sp = pool.tile([P, F], f32)
            sg = pool.tile([P, F], f32)
            v0 = pool.tile([P, F], f32)
            v128 = pool.tile([P, F], f32)
            t1 = pool.tile([P, F], f32)
            r = pool.tile([P, F], f32)
            u = pool.tile([P, F], f32)
            g = pool.tile([P, F], f32)
            stats = pool.tile([P, 4], f32)

            # single input DMA: only the first op on each engine carries the
            # DMA wait (HW allows one sync wait per regular instruction).
            # mask on GpSimd overlaps the DVE chain head.
            nc.gpsimd.tensor_scalar(mask[:], tt[:], 128.0, None, Op.is_equal)
            nc.vector.tensor_scalar(sp[:], tp[:], 128.0, None, Op.mult)
            nc.vector.tensor_scalar(sg[:], tg[:], 128.0, None, Op.mult)
            nc.vector.tensor_tensor(v0[:], sp[:], sg[:], op=Op.min)
            nc.vector.tensor_tensor(v128[:], v0[:], mask[:], op=Op.mult)
            # t1 = RN(RN(v128*SCALE) + MAGIC); r = t1 - MAGIC  (exact RNE)
            nc.vector.tensor_scalar(t1[:], v128[:], SCALE, MAGIC, Op.mult, Op.add)
            nc.vector.tensor_scalar(r[:], t1[:], MAGIC, None, Op.subtract)
            nc.vector.tensor_scalar(u[:], r[:], C128, None, Op.mult)
            nc.vector.tensor_tensor(g[:], v128[:], u[:], op=Op.is_gt)

            nc.vector.tensor_reduce(stats[:, 0:1], r[:], mybir.AxisListType.X, Op.add)
            nc.vector.tensor_reduce(stats[:, 1:2], g[:], mybir.AxisListType.X, Op.add)
            nc.vector.tensor_reduce(stats[:, 2:3], mask[:], mybir.AxisListType.X, Op.add)

            nc.sync.dma_start(out[:], stats[:, 0:3])

    _split_multi_waits(nc, mybir)
    return nc


def _split_multi_waits(nc, mybir):
    """walrus codegen allows only one sync wait per regular instruction.

    Tile's kernel-tail drain waits on every DMA-queue semaphore plus the
    compute tick at once.  Hoist all but the last wait of any multi-wait
    instruction onto dedicated InstEventSemaphore instructions (which support
    waits) placed immediately before it on the same engine - semantically
    identical, since the engine executes them in order.
    """
    n = 0
    for bb in nc.main_func.blocks:
        new_insts = []
        for ins in bb.instructions:
            si = getattr(ins, "sync_info", None)
            if (
                si is not None
                and si.on_wait
                and len(si.on_wait) > 1
                and not isinstance(ins, mybir.InstEventSemaphore)
            ):
                for wt in si.on_wait[:-1]:
                    ev = mybir.InstEventSemaphore(
                        name=f"waitsplit-{n}", ins=[], outs=[]
                    )
                    n += 1
                    ev.engine = ins.engine
                    ev.sync_info = mybir.SyncInfo(on_wait=[wt], on_update=[])
                    nc.register_instruction(ev, overwrite=True)
                    new_insts.append(ev)
                si.on_wait = si.on_wait[-1:]
            new_insts.append(ins)
        bb.instructions[:] = new_insts


def _get_nc():
    if "nc" not in _CACHE:
        _CACHE["nc"] = _build()
    return _CACHE["nc"]


def _shard(x):
    return np.ascontiguousarray(x.reshape(N_CORES, P, F))


def _pack(ap, ag, tm):
    """Per-core (P, 3F) int32: [pred bits | gt bits | trimap]."""
    packed = np.empty((N_CORES, P, 3 * F), dtype=np.int32)
    packed[:, :, 0:F] = _shard(ap).view(np.int32)
    packed[:, :, F:2 * F] = _shard(ag).view(np.int32)
    packed[:, :, 2 * F:3 * F] = _shard(tm)
    return packed


def kernel(alpha_pred, alpha_gt, trimap):
    from concourse.bass_utils import run_bass_kernel_spmd

    ap = np.ascontiguousarray(alpha_pred, dtype=np.float32)
    ag = np.ascontiguousarray(alpha_gt, dtype=np.float32)
    tm = np.ascontiguousarray(trimap, dtype=np.int32)
    assert ap.size == TOTAL and ag.size == TOTAL and tm.size == TOTAL

    packed = _pack(ap, ag, tm)
    in_maps = [{"packed": packed[i]} for i in range(N_CORES)]

    nc = _get_nc()
    res = run_bass_kernel_spmd(nc, in_maps, list(range(N_CORES))).results

    s_cnt = 0.0
    s_msk = 0.0
    for i in range(N_CORES):
        st = res[i]["stats"].astype(np.float64)
        s_cnt += float(st[:, 0].sum() + st[:, 1].sum())
        s_msk += float(st[:, 2].sum())

    # loss = sum(mask * (101 - cnt)/101) / (sum(mask) + 1e-8), in fp32 like ref
    num = np.float32((101.0 * s_msk - s_cnt) / 101.0)
    den = np.float32(np.float32(s_msk) + np.float32(1e-8))
    return np.asarray(num / den, dtype=np.float32)


# revision 17
# speedup vs baseline: 1.1898x; 1.1898x over previous
"""ConnectivityLoss kernel for Trainium2 (Bass/Tile), 8-core data-parallel.

Math: the reference's 32-step 3x3 max-dilation chain cancels algebraically.
For binary maps, dilation D(x) >= x pointwise (3x3 SAME window contains the
center), so pred_bin * D32(gt_bin) * gt_bin * D32(pred_bin) == pred_bin * gt_bin
for every pixel: whenever both center bits are 1 the two dilations are 1 at
that pixel too, and otherwise the product is 0 regardless.  Hence

    match[b,k,i,j] = (alpha_pred > t_k) * (alpha_gt > t_k)
                   = (min(alpha_pred, alpha_gt) > t_k)

    err_px = (101 - cnt) / 101      with cnt = #{k in 0..100 : t_k < m},
                                    m = min(alpha_pred, alpha_gt)
    loss   = sum(err_px * [trimap == 128]) / (sum([trimap == 128]) + 1e-8)

cnt is evaluated per pixel with an exactly-rounded two-scale trick instead of
101 compares (t_k = RN(k * fp32(0.01)), the jnp.arange values):

    v128 = m * 128                  exact (power of two)
    v    = v128 * 0.78125           = RN(m * 100)   (0.78125 = 100/128 exact)
    r    = RN(RN(v + 2^23) - 2^23)  = round-to-nearest-int(v), candidate bin
    u    = r * 1.28                 = RN(r * (fp32(0.01)*128)) = t_r * 128
                                      exactly (x128 commutes with rounding)
    cnt  = r + [v128 > u]           since t_r < m <=> u < v128 exactly, and
                                    r is within 0.5 of m*100 so the count can
                                    only be r or r+1.

Verified exhaustively against the reference predicate for ALL 2^30 fp32
values in [0,1): zero mismatches.  m = 0 (unmasked pixels zeroed by the mask
multiply) gives r = 0, u = 0, [0 > 0] = 0 -> cnt = 0 as required.

Sharding: data-parallel over flattened B*H*W pixels, 8 equal contiguous
shards of 32768 = 128x256 elements; each core reduces its shard to
per-partition partial sums (sum r, sum g, sum mask), host combines scalars.
"""

import numpy as np

N_CORES = 8
P = 128          # SBUF partitions
F = 256          # free dim; per-core shard = P*F = 32768 pixels
TOTAL = 4 * 1 * 256 * 256

MAGIC = 8388608.0                       # 2^23
C128 = float(np.float32(0.01) * np.float32(128.0))   # 1.28 in fp32, exact
SCALE = 0.78125                          # 100/128, exact in fp32

_CACHE = {}


def _build():
    import concourse.bass as bass
    import concourse.tile as tile
    from concourse import mybir

    f32 = mybir.dt.float32
    i32 = mybir.dt.int32
    Op = mybir.AluOpType

    nc = bass.Bass(
        "TRN2",
        target_bir_lowering=False,
        debug=False,
        enable_asserts=False,
        num_devices=N_CORES,
    )
    pred = nc.dram_tensor("pred", [P, F], f32, kind="ExternalInput")
    gt = nc.dram_tensor("gt", [P, F], f32, kind="ExternalInput")
    tri = nc.dram_tensor("tri", [P, F], i32, kind="ExternalInput")
    out = nc.dram_tensor("stats", [P, 3], f32, kind="ExternalOutput")

    with tile.TileContext(nc) as tc:
        with tc.tile_pool(name="pool", bufs=1) as pool:
            tp = pool.tile([P, F], f32)
            tg = pool.tile([P, F], f32)
            tt = pool.tile([P, F], i32)
            # three DMA-capable engines (SP + ACT hwdge, Pool swdge) so the
            # three loads issue and fly concurrently
            nc.sync.dma_start(tp[:], pred[:])
            nc.scalar.dma_start(tg[:], gt[:])
            nc.gpsimd.dma_start(tt[:], tri[:])

            mask = pool.tile([P, F], f32)
            sp = pool.tile([P, F], f32)
            sg = pool.tile([P, F], f32)
            v0 = pool.tile([P, F], f32)
            v128 = pool.tile([P, F], f32)
            t1 = pool.tile([P, F], f32)
            r = pool.tile([P, F], f32)
            u = pool.tile([P, F], f32)
            g = pool.tile([P, F], f32)
            stats = pool.tile([P, 4], f32)

            # HW allows one sync wait per instruction and dependent DVE ops
            # spend it on the DVE self-semaphore, so each DMA'd tile is first
            # touched by an op with no DVE deps (min commutes with the
            # monotone exact x*128).
            nc.vector.tensor_scalar(mask[:], tt[:], 128.0, None, Op.is_equal)
            nc.vector.tensor_scalar(sp[:], tp[:], 128.0, None, Op.mult)
            nc.vector.tensor_scalar(sg[:], tg[:], 128.0, None, Op.mult)
            nc.vector.tensor_tensor(v0[:], sp[:], sg[:], op=Op.min)
            nc.vector.tensor_tensor(v128[:], v0[:], mask[:], op=Op.mult)
            # t1 = RN(RN(v128*SCALE) + MAGIC); r = t1 - MAGIC  (exact RNE)
            nc.vector.tensor_scalar(t1[:], v128[:], SCALE, MAGIC, Op.mult, Op.add)
            nc.vector.tensor_scalar(r[:], t1[:], MAGIC, None, Op.subtract)
            nc.vector.tensor_scalar(u[:], r[:], C128, None, Op.mult)
            nc.vector.tensor_tensor(g[:], v128[:], u[:], op=Op.is_gt)

            nc.vector.tensor_reduce(stats[:, 0:1], r[:], mybir.AxisListType.X, Op.add)
            nc.vector.tensor_reduce(stats[:, 1:2], g[:], mybir.AxisListType.X, Op.add)
            nc.vector.tensor_reduce(stats[:, 2:3], mask[:], mybir.AxisListType.X, Op.add)

            nc.sync.dma_start(out[:], stats[:, 0:3])

    _split_multi_waits(nc, mybir)
    return nc


def _split_multi_waits(nc, mybir):
    """walrus codegen allows only one sync wait per regular instruction.

    Tile's kernel-tail drain waits on every DMA-queue semaphore plus the
    compute tick at once.  Hoist all but the last wait of any multi-wait
    instruction onto dedicated InstEventSemaphore instructions (which support
    waits) placed immediately before it on the same engine - semantically
    identical, since the engine executes them in order.
    """
    n = 0
    for bb in nc.main_func.blocks:
        new_insts = []
        for ins in bb.instructions:
            si = getattr(ins, "sync_info", None)
            if (
                si is not None
                and si.on_wait
                and len(si.on_wait) > 1
                and not isinstance(ins, mybir.InstEventSemaphore)
            ):
                for wt in si.on_wait[:-1]:
                    ev = mybir.InstEventSemaphore(
                        name=f"waitsplit-{n}", ins=[], outs=[]
                    )
                    n += 1
                    ev.engine = ins.engine
                    ev.sync_info = mybir.SyncInfo(on_wait=[wt], on_update=[])
                    nc.register_instruction(ev, overwrite=True)
                    new_insts.append(ev)
                si.on_wait = si.on_wait[-1:]
            new_insts.append(ins)
        bb.instructions[:] = new_insts


def _get_nc():
    if "nc" not in _CACHE:
        _CACHE["nc"] = _build()
    return _CACHE["nc"]


def _shard(x):
    return np.ascontiguousarray(x.reshape(N_CORES, P, F))


def _pack(ap, ag, tm):
    """Per-core input maps for the three DRAM parameters."""
    aps, ags, tms = _shard(ap), _shard(ag), _shard(tm)
    return [
        {"pred": aps[i], "gt": ags[i], "tri": tms[i]} for i in range(N_CORES)
    ]


def kernel(alpha_pred, alpha_gt, trimap):
    from concourse.bass_utils import run_bass_kernel_spmd

    ap = np.ascontiguousarray(alpha_pred, dtype=np.float32)
    ag = np.ascontiguousarray(alpha_gt, dtype=np.float32)
    tm = np.ascontiguousarray(trimap, dtype=np.int32)
    assert ap.size == TOTAL and ag.size == TOTAL and tm.size == TOTAL

    in_maps = _pack(ap, ag, tm)

    nc = _get_nc()
    res = run_bass_kernel_spmd(nc, in_maps, list(range(N_CORES))).results

    s_cnt = 0.0
    s_msk = 0.0
    for i in range(N_CORES):
        st = res[i]["stats"].astype(np.float64)
        s_cnt += float(st[:, 0].sum() + st[:, 1].sum())
        s_msk += float(st[:, 2].sum())

    # loss = sum(mask * (101 - cnt)/101) / (sum(mask) + 1e-8), in fp32 like ref
    num = np.float32((101.0 * s_msk - s_cnt) / 101.0)
    den = np.float32(np.float32(s_msk) + np.float32(1e-8))
    return np.asarray(num / den, dtype=np.float32)


# revision 19
# speedup vs baseline: 1.2155x; 1.0215x over previous
"""ConnectivityLoss kernel for Trainium2 (Bass/Tile), 8-core data-parallel.

Math: the reference's 32-step 3x3 max-dilation chain cancels algebraically.
For binary maps, dilation D(x) >= x pointwise (3x3 SAME window contains the
center), so pred_bin * D32(gt_bin) * gt_bin * D32(pred_bin) == pred_bin * gt_bin
for every pixel: whenever both center bits are 1 the two dilations are 1 at
that pixel too, and otherwise the product is 0 regardless.  Hence

    match[b,k,i,j] = (alpha_pred > t_k) * (alpha_gt > t_k)
                   = (min(alpha_pred, alpha_gt) > t_k)

    err_px = (101 - cnt) / 101      with cnt = #{k in 0..100 : t_k < m},
                                    m = min(alpha_pred, alpha_gt)
    loss   = sum(err_px * [trimap == 128]) / (sum([trimap == 128]) + 1e-8)

cnt is evaluated per pixel with an exactly-rounded two-scale trick instead of
101 compares (t_k = RN(k * fp32(0.01)), the jnp.arange values):

    v128 = m * 128                  exact (power of two)
    v    = v128 * 0.78125           = RN(m * 100)   (0.78125 = 100/128 exact)
    r    = RN(RN(v + 2^23) - 2^23)  = round-to-nearest-int(v), candidate bin
    u    = r * 1.28                 = RN(r * (fp32(0.01)*128)) = t_r * 128
                                      exactly (x128 commutes with rounding)
    cnt  = r + [v128 > u]           since t_r < m <=> u < v128 exactly, and
                                    r is within 0.5 of m*100 so the count can
                                    only be r or r+1.

Verified exhaustively against the reference predicate for ALL 2^30 fp32
values in [0,1): zero mismatches.  m = 0 (unmasked pixels zeroed by the mask
multiply) gives r = 0, u = 0, [0 > 0] = 0 -> cnt = 0 as required.

Sharding: data-parallel over flattened B*H*W pixels, 8 equal contiguous
shards of 32768 = 128x256 elements; each core reduces its shard to
per-partition partial sums (sum r, sum g, sum mask), host combines scalars.
"""

import numpy as np

N_CORES = 8
P = 128          # SBUF partitions
F = 256          # free dim; per-core shard = P*F = 32768 pixels
TOTAL = 4 * 1 * 256 * 256

MAGIC = 8388608.0                       # 2^23
C128 = float(np.float32(0.01) * np.float32(128.0))   # 1.28 in fp32, exact
SCALE = 0.78125                          # 100/128, exact in fp32

_CACHE = {}


def _build():
    import concourse.bass as bass
    import concourse.tile as tile
    from concourse import mybir

    f32 = mybir.dt.float32
    i32 = mybir.dt.int32
    Op = mybir.AluOpType

    nc = bass.Bass(
        "TRN2",
        target_bir_lowering=False,
        debug=False,
        enable_asserts=False,
        num_devices=N_CORES,
    )
    pred = nc.dram_tensor("pred", [P, F], f32, kind="ExternalInput")
    gt = nc.dram_tensor("gt", [P, F], f32, kind="ExternalInput")
    tri = nc.dram_tensor("tri", [P, F], i32, kind="ExternalInput")
    out = nc.dram_tensor("stats", [P, 3], f32, kind="ExternalOutput")

    with tile.TileContext(nc) as tc:
        with tc.tile_pool(name="pool", bufs=1) as pool:
            tp = pool.tile([P, F], f32)
            tg = pool.tile([P, F], f32)
            tt = pool.tile([P, F], i32)
            # three DMA-capable engines (SP + ACT hwdge, Pool swdge) so the
            # three loads issue and fly concurrently
            nc.sync.dma_start(tp[:], pred[:])
            nc.scalar.dma_start(tg[:], gt[:])
            nc.gpsimd.dma_start(tt[:], tri[:])

            mask = pool.tile([P, F], f32)
            sp = pool.tile([P, F], f32)
            sg = pool.tile([P, F], f32)
            v0 = pool.tile([P, F], f32)
            v128 = pool.tile([P, F], f32)
            t1 = pool.tile([P, F], f32)
            r = pool.tile([P, F], f32)
            u = pool.tile([P, F], f32)
            g = pool.tile([P, F], f32)
            stats = pool.tile([P, 4], f32)

            # HW allows one sync wait per instruction and dependent DVE ops
            # spend it on the DVE self-semaphore, so each DMA'd tile is first
            # touched by an op with no DVE deps (min commutes with the
            # monotone exact x*128).  mask comes after min: the slower SWDGE
            # trimap load hides behind the first three DVE ops.  The three
            # row-sums ride the accum_out port of the producing ops, so no
            # separate reduce instructions are needed.
            nc.vector.tensor_scalar(sp[:], tp[:], 128.0, None, Op.mult)
            nc.vector.tensor_scalar(sg[:], tg[:], 128.0, None, Op.mult)
            nc.vector.tensor_tensor(v0[:], sp[:], sg[:], op=Op.min)
            # mask = (tri == 128) bypass sp; accum -> row-sum(mask)
            nc.vector.scalar_tensor_tensor(
                mask[:], tt[:], 128.0, sp[:], op0=Op.is_equal, op1=Op.bypass,
                accum_out=stats[:, 2:3],
            )
            nc.vector.tensor_tensor(v128[:], v0[:], mask[:], op=Op.mult)
            # t1 = RN(RN(v128*SCALE) + MAGIC); r = t1 - MAGIC  (exact RNE)
            nc.vector.tensor_scalar(t1[:], v128[:], SCALE, MAGIC, Op.mult, Op.add)
            # r with accum -> row-sum(r)  (op1 doubles as the reduce op)
            nc.vector.tensor_scalar(
                r[:], t1[:], MAGIC, None, Op.subtract, Op.add,
                accum_out=stats[:, 0:1],
            )
            nc.vector.tensor_scalar(u[:], r[:], C128, None, Op.mult)
            # g = (u bypass 0) < v128 = [t_r*128 < m*128]; accum -> row-sum(g)
            nc.vector.scalar_tensor_tensor(
                g[:], u[:], 0.0, v128[:], op0=Op.bypass, op1=Op.is_lt,
                accum_out=stats[:, 1:2],
            )

            nc.sync.dma_start(out[:], stats[:, 0:3])

    _split_multi_waits(nc, mybir)
    return nc


def _split_multi_waits(nc, mybir):
    """walrus codegen allows only one sync wait per regular instruction.

    Tile's kernel-tail drain waits on every DMA-queue semaphore plus the
    compute tick at once.  Hoist all but the last wait of any multi-wait
    instruction onto dedicated InstEventSemaphore instructions (which support
    waits) placed immediately before it on the same engine - semantically
    identical, since the engine executes them in order.
    """
    n = 0
    for bb in nc.main_func.blocks:
        new_insts = []
        for ins in bb.instructions:
            si = getattr(ins, "sync_info", None)
            if (
                si is not None
                and si.on_wait
                and len(si.on_wait) > 1
                and not isinstance(ins, mybir.InstEventSemaphore)
            ):
                for wt in si.on_wait[:-1]:
                    ev = mybir.InstEventSemaphore(
                        name=f"waitsplit-{n}", ins=[], outs=[]
                    )
                    n += 1
                    ev.engine = ins.engine
                    ev.sync_info = mybir.SyncInfo(on_wait=[wt], on_update=[])
                    nc.register_instruction(ev, overwrite=True)
                    new_insts.append(ev)
                si.on_wait = si.on_wait[-1:]
            new_insts.append(ins)
        bb.instructions[:] = new_insts


def _get_nc():
    if "nc" not in _CACHE:
        _CACHE["nc"] = _build()
    return _CACHE["nc"]


def _shard(x):
    return np.ascontiguousarray(x.reshape(N_CORES, P, F))


def _pack(ap, ag, tm):
    """Per-core input maps for the three DRAM parameters."""
    aps, ags, tms = _shard(ap), _shard(ag), _shard(tm)
    return [
        {"pred": aps[i], "gt": ags[i], "tri": tms[i]} for i in range(N_CORES)
    ]


def kernel(alpha_pred, alpha_gt, trimap):
    from concourse.bass_utils import run_bass_kernel_spmd

    ap = np.ascontiguousarray(alpha_pred, dtype=np.float32)
    ag = np.ascontiguousarray(alpha_gt, dtype=np.float32)
    tm = np.ascontiguousarray(trimap, dtype=np.int32)
    assert ap.size == TOTAL and ag.size == TOTAL and tm.size == TOTAL

    in_maps = _pack(ap, ag, tm)

    nc = _get_nc()
    res = run_bass_kernel_spmd(nc, in_maps, list(range(N_CORES))).results

    s_cnt = 0.0
    s_msk = 0.0
    for i in range(N_CORES):
        st = res[i]["stats"].astype(np.float64)
        s_cnt += float(st[:, 0].sum() + st[:, 1].sum())
        s_msk += float(st[:, 2].sum())

    # loss = sum(mask * (101 - cnt)/101) / (sum(mask) + 1e-8), in fp32 like ref
    num = np.float32((101.0 * s_msk - s_cnt) / 101.0)
    den = np.float32(np.float32(s_msk) + np.float32(1e-8))
    return np.asarray(num / den, dtype=np.float32)


# revision 21
# speedup vs baseline: 1.2701x; 1.0450x over previous
"""ConnectivityLoss kernel for Trainium2 (Bass/Tile), 8-core data-parallel.

Math: the reference's 32-step 3x3 max-dilation chain cancels algebraically.
For binary maps, dilation D(x) >= x pointwise (3x3 SAME window contains the
center), so pred_bin * D32(gt_bin) * gt_bin * D32(pred_bin) == pred_bin * gt_bin
for every pixel: whenever both center bits are 1 the two dilations are 1 at
that pixel too, and otherwise the product is 0 regardless.  Hence

    match[b,k,i,j] = (alpha_pred > t_k) * (alpha_gt > t_k)
                   = (min(alpha_pred, alpha_gt) > t_k)

    err_px = (101 - cnt) / 101      with cnt = #{k in 0..100 : t_k < m},
                                    m = min(alpha_pred, alpha_gt)
    loss   = sum(err_px * [trimap == 128]) / (sum([trimap == 128]) + 1e-8)

cnt is evaluated per pixel with an exactly-rounded two-scale trick instead of
101 compares (t_k = RN(k * fp32(0.01)), the jnp.arange values):

    v128 = m * 128                  exact (power of two)
    v    = v128 * 0.78125           = RN(m * 100)   (0.78125 = 100/128 exact)
    r    = RN(RN(v + 2^23) - 2^23)  = round-to-nearest-int(v), candidate bin
    u    = r * 1.28                 = RN(r * (fp32(0.01)*128)) = t_r * 128
                                      exactly (x128 commutes with rounding)
    cnt  = r + [v128 > u]           since t_r < m <=> u < v128 exactly, and
                                    r is within 0.5 of m*100 so the count can
                                    only be r or r+1.

Verified exhaustively against the reference predicate for ALL 2^30 fp32
values in [0,1): zero mismatches.  m = 0 (unmasked pixels zeroed by the mask
multiply) gives r = 0, u = 0, [0 > 0] = 0 -> cnt = 0 as required.

Sharding: data-parallel over flattened B*H*W pixels, 8 equal contiguous
shards of 32768 = 128x256 elements; each core reduces its shard to
per-partition partial sums (sum r, sum g, sum mask), host combines scalars.
"""

import numpy as np

N_CORES = 8
P = 128          # SBUF partitions
F = 256          # free dim; per-core shard = P*F = 32768 pixels
TOTAL = 4 * 1 * 256 * 256

MAGIC = 8388608.0                       # 2^23
C128 = float(np.float32(0.01) * np.float32(128.0))   # 1.28 in fp32, exact
SCALE = 0.78125                          # 100/128, exact in fp32

_CACHE = {}


def _build():
    import concourse.bass as bass
    import concourse.tile as tile
    from concourse import mybir

    f32 = mybir.dt.float32
    i32 = mybir.dt.int32
    Op = mybir.AluOpType

    nc = bass.Bass(
        "TRN2",
        target_bir_lowering=False,
        debug=False,
        enable_asserts=False,
        num_devices=N_CORES,
    )
    u8 = mybir.dt.uint8
    pred = nc.dram_tensor("pred", [P, F], f32, kind="ExternalInput")
    gt = nc.dram_tensor("gt", [P, F], f32, kind="ExternalInput")
    tri = nc.dram_tensor("tri", [P, F], u8, kind="ExternalInput")
    out = nc.dram_tensor("stats", [P, 3], f32, kind="ExternalOutput")

    with tile.TileContext(nc) as tc:
        with tc.tile_pool(name="pool", bufs=1) as pool:
            tp = pool.tile([P, F], f32)
            tg = pool.tile([P, F], f32)
            tt = pool.tile([P, F], u8)
            # two HWDGE queues (SP + ACT); the small uint8 trimap rides the
            # SP queue behind pred.  No SWDGE: its ring drain costs ~2.7us.
            nc.sync.dma_start(tp[:], pred[:])
            nc.scalar.dma_start(tg[:], gt[:])
            nc.sync.dma_start(tt[:], tri[:])

            mask = pool.tile([P, F], f32)
            sp = pool.tile([P, F], f32)
            sg = pool.tile([P, F], f32)
            v0 = pool.tile([P, F], f32)
            v128 = pool.tile([P, F], f32)
            t1 = pool.tile([P, F], f32)
            r = pool.tile([P, F], f32)
            g = pool.tile([P, F], f32)
            stats = pool.tile([P, 4], f32)

            # HW allows one sync wait per instruction and dependent DVE ops
            # spend it on the DVE self-semaphore, so each DMA'd tile is first
            # touched by an op with no DVE deps (min commutes with the
            # monotone exact x*128).  mask comes after min so the second SP
            # transfer hides behind the first DVE ops.  The three row-sums
            # ride the accum_out port of the producing ops - no separate
            # reduce instructions.
            nc.vector.tensor_scalar(sp[:], tp[:], 128.0, None, Op.mult)
            nc.vector.tensor_scalar(sg[:], tg[:], 128.0, None, Op.mult)
            nc.vector.tensor_tensor(v0[:], sp[:], sg[:], op=Op.min)
            # mask = (tri == 128) bypass sp; accum -> row-sum(mask)
            nc.vector.scalar_tensor_tensor(
                mask[:], tt[:], 128.0, sp[:], op0=Op.is_equal, op1=Op.bypass,
                accum_out=stats[:, 2:3],
            )
            nc.vector.tensor_tensor(v128[:], v0[:], mask[:], op=Op.mult)
            # t1 = RN(RN(v128*SCALE) + MAGIC); r = t1 - MAGIC  (exact RNE)
            nc.vector.tensor_scalar(t1[:], v128[:], SCALE, MAGIC, Op.mult, Op.add)
            # r with accum -> row-sum(r)  (op1 doubles as the reduce op)
            nc.vector.tensor_scalar(
                r[:], t1[:], MAGIC, None, Op.subtract, Op.add,
                accum_out=stats[:, 0:1],
            )
            # g = (RN(r*C128) < v128) = [t_r*128 < m*128]; accum -> row-sum(g)
            nc.vector.scalar_tensor_tensor(
                g[:], r[:], C128, v128[:], op0=Op.mult, op1=Op.is_lt,
                accum_out=stats[:, 1:2],
            )

            nc.sync.dma_start(out[:], stats[:, 0:3])

    _split_multi_waits(nc, mybir)
    return nc


def _split_multi_waits(nc, mybir):
    """walrus codegen allows only one sync wait per regular instruction.

    Tile's kernel-tail drain waits on every DMA-queue semaphore plus the
    compute tick at once.  Hoist all but the last wait of any multi-wait
    instruction onto dedicated InstEventSemaphore instructions (which support
    waits) placed immediately before it on the same engine - semantically
    identical, since the engine executes them in order.
    """
    n = 0
    for bb in nc.main_func.blocks:
        new_insts = []
        for ins in bb.instructions:
            si = getattr(ins, "sync_info", None)
            if (
                si is not None
                and si.on_wait
                and len(si.on_wait) > 1
                and not isinstance(ins, mybir.InstEventSemaphore)
            ):
                for wt in si.on_wait[:-1]:
                    ev = mybir.InstEventSemaphore(
                        name=f"waitsplit-{n}", ins=[], outs=[]
                    )
                    n += 1
                    ev.engine = ins.engine
                    ev.sync_info = mybir.SyncInfo(on_wait=[wt], on_update=[])
                    nc.register_instruction(ev, overwrite=True)
                    new_insts.append(ev)
                si.on_wait = si.on_wait[-1:]
            new_insts.append(ins)
        bb.instructions[:] = new_insts


def _get_nc():
    if "nc" not in _CACHE:
        _CACHE["nc"] = _build()
    return _CACHE["nc"]


def _shard(x):
    return np.ascontiguousarray(x.reshape(N_CORES, P, F))


def _pack(ap, ag, tm):
    """Per-core input maps; trimap values are 0..255 so uint8 is lossless."""
    aps, ags = _shard(ap), _shard(ag)
    tms = np.ascontiguousarray(_shard(tm).astype(np.uint8))
    return [
        {"pred": aps[i], "gt": ags[i], "tri": tms[i]} for i in range(N_CORES)
    ]


def kernel(alpha_pred, alpha_gt, trimap):
    from concourse.bass_utils import run_bass_kernel_spmd

    ap = np.ascontiguousarray(alpha_pred, dtype=np.float32)
    ag = np.ascontiguousarray(alpha_gt, dtype=np.float32)
    tm = np.ascontiguousarray(trimap, dtype=np.int32)
    assert ap.size == TOTAL and ag.size == TOTAL and tm.size == TOTAL

    in_maps = _pack(ap, ag, tm)

    nc = _get_nc()
    res = run_bass_kernel_spmd(nc, in_maps, list(range(N_CORES))).results

    s_cnt = 0.0
    s_msk = 0.0
    for i in range(N_CORES):
        st = res[i]["stats"].astype(np.float64)
        s_cnt += float(st[:, 0].sum() + st[:, 1].sum())
        s_msk += float(st[:, 2].sum())

    # loss = sum(mask * (101 - cnt)/101) / (sum(mask) + 1e-8), in fp32 like ref
    num = np.float32((101.0 * s_msk - s_cnt) / 101.0)
    den = np.float32(np.float32(s_msk) + np.float32(1e-8))
    return np.asarray(num / den, dtype=np.float32)
